# revision 33
# baseline (speedup 1.0000x reference)
"""Trainium2 Bass kernel for nn_ActorNetwork (gnn_message_passing).

Mathematical collapse (verified vs reference to ~2.5e-8 rel): the reference
broadcasts edge_index as ``broadcast_to(ei[None], (B,2,E)).reshape(2,-1)``,
making row == col elementwise -> every edge is a self-loop and the GCN
normalization cancels exactly: ``gcn_conv(x, W, b) == x @ W + b``.  The
network is two dense layers + softmax over nodes, plus a per-(node,k)
2-layer MLP + softmax over k.  ``edge_index`` never ships to the device.
Scalar biases bfc / bc2 are constant softmax shifts and cancel.

Device strategy (data-parallel over batch, core i = graphs 2i, 2i+1):
 - Host pre-transposes to feature-major so every DMA is contiguous.
 - Inputs in fp8e4m3 (measured end-to-end rel err ~7.9e-3 vs 2e-2 gate):
   4.48 MB/core of input traffic.  Intermediates bf16, accumulation f32.
 - 20 chunks of 500 nodes, 4 chunks per PSUM tile (partition slots
   0/32/64/96) so elementwise engines see FD=500, 128-partition tiles.
 - Node + col work interleaved per group to keep TensorE dense; the
   softmax normalization (needs the global node sum) runs as a short
   finalize pass.
 - 1/D via ScalarE Ln->Exp (exp & ln share one ACT table set) instead of
   the slow iterative DVE reciprocal.
"""

import numpy as np

B, N, F, K, FC = 16, 5000, 128, 10, 32
NCORES = 8
GPC = B // NCORES          # graphs per core = 2
M = GPC * N                # nodes per core = 10000
CH = 500                   # chunk size (nodes)
NCHUNK = M // CH           # 20
GRP = 4                    # chunks per PSUM group
NGRP = NCHUNK // GRP       # 5
CPG = N // CH              # chunks per graph = 10

# column maps for the three packed constant tensors
_W8, _WBF, _W32 = {}, {}, {}
def _mk(dct, *spec):
    off = 0
    for name, width in spec:
        dct[name] = (off, off + width)
        off += width
    return off
_NW8 = _mk(_W8, ("w1p", 32), ("wc1b", 64), ("wc1b2p", 64), ("pad8", 352))
_NWB = _mk(_WBF, ("w2b", 64), ("wfcb", 4), ("wc2a", 32), ("wc2b", 32),
           ("kmask", 4), ("padb", 120))
_NW32 = _mk(_W32, ("bmask", 128), ("smsk", 2 * NGRP), ("smap", 4 * NGRP),
            ("b1r", 1), ("b2r", 1), ("bc1r", 1))


def _pack_consts(W1, b1, W2, b2, Wfc, Wc1, bc1, Wc2):
    import ml_dtypes
    w8 = np.zeros((128, _NW8), np.float32)
    wb = np.zeros((128, _NWB), np.float32)
    w3 = np.zeros((128, _NW32), np.float32)

    lo = _W8["w1p"][0]
    w8[:, lo:lo + 16] = W1
    lo = _W8["wc1b"][0]
    for a in range(4):
        w8[32 * a:32 * a + 32, lo + 16 * a:lo + 16 * a + 16] = Wc1
    lo = _W8["wc1b2p"][0]
    for s in range(2):          # chunk parity within a pair
        for t in range(2):      # k = 8 + t
            r = 64 * s + 32 * t
            w8[r:r + 32, lo + 32 * s + 16 * t:lo + 32 * s + 16 * t + 16] = Wc1

    lo = _WBF["w2b"][0]
    for j in range(4):
        wb[32 * j:32 * j + 16, lo + 16 * j:lo + 16 * j + 16] = W2
    lo = _WBF["wfcb"][0]
    for j in range(4):
        wb[16 * j:16 * j + 16, lo + j] = Wfc[:, 0]
    lo = _WBF["wc2a"][0]
    for k in range(8):
        wb[16 * k:16 * k + 16, lo + k] = Wc2[:, 0]
    lo = _WBF["wc2b"][0]
    for j in range(4):
        for t in range(2):
            r = 32 * j + 16 * t
            wb[r:r + 16, lo + 8 + t] = Wc2[:, 0]
    lo = _WBF["kmask"][0]
    for j in range(4):
        wb[32 * j:32 * j + 10, lo + j] = 1.0

    lo = _W32["bmask"][0]
    for j in range(4):
        w3[j, lo + 32 * j:lo + 32 * j + 10] = 1.0
    lo = _W32["smsk"][0]
    for c in range(NGRP):
        for j in range(GRP):
            g = (GRP * c + j) // CPG
            w3[j, lo + 2 * c + g] = 1.0
    lo = _W32["smap"][0]
    for c in range(NGRP):
        for j in range(GRP):
            g = (GRP * c + j) // CPG
            w3[g, lo + 4 * c + j] = 1.0
    lo = _W32["b1r"][0]
    for j in range(4):
        w3[32 * j:32 * j + 16, lo] = b1
    lo = _W32["b2r"][0]
    for j in range(4):
        w3[16 * j:16 * j + 16, lo] = b2
    lo = _W32["bc1r"][0]
    for s in range(8):
        w3[16 * s:16 * s + 16, lo] = bc1
    return (w8.astype(ml_dtypes.float8_e4m3), wb.astype(ml_dtypes.bfloat16),
            w3)


_CACHED = None


def _build():
    """Build + bacc-compile the per-core Bass graph (same on all 8 cores)."""
    from contextlib import ExitStack

    import concourse.tile as tile
    from concourse import bacc, mybir

    f32 = mybir.dt.float32
    bf16 = mybir.dt.bfloat16
    f8 = mybir.dt.float8e4
    AF = mybir.ActivationFunctionType
    ALU = mybir.AluOpType

    import concourse.bacc as bacc_mod
    _orig_gat = bacc_mod.get_activation_tables

    def _gat_one_set(arch):
        # keep every entry (act_func_set_id is positional) but empty all
        # sets except the one that covers Relu+Exp+Ln, forcing its choice
        t = _orig_gat(arch)
        if "natural_log_exp_and_others" not in t:
            return t
        return {k: (v if k == "natural_log_exp_and_others" else set())
                for k, v in t.items()}

    bacc_mod.get_activation_tables = _gat_one_set

    nc = bacc.Bacc("TRN2", target_bir_lowering=False, debug=False,
                   num_devices=NCORES)

    xt_p = nc.dram_tensor("xt", [128, M], f8, kind="ExternalInput").ap()
    ct_p = nc.dram_tensor("ct", [256, M], f8, kind="ExternalInput").ap()
    c2_p = nc.dram_tensor("c2", [128, M // 2], f8, kind="ExternalInput").ap()
    w8_p = nc.dram_tensor("w8", [128, _NW8], f8, kind="ExternalInput").ap()
    wb_p = nc.dram_tensor("wb", [128, _NWB], bf16, kind="ExternalInput").ap()
    w3_p = nc.dram_tensor("w3", [128, _NW32], f32, kind="ExternalInput").ap()
    out_p = nc.dram_tensor("out", [128, NGRP * CH], f32,
                       kind="ExternalOutput").ap()

    with tile.TileContext(nc) as tc, ExitStack() as ctx:
        wpool = ctx.enter_context(tc.tile_pool(name="wc", bufs=1))
        wt8 = wpool.tile([128, _NW8], f8, tag="wt8")
        nc.sync.dma_start(out=wt8[:], in_=w8_p[:])
        wtb = wpool.tile([128, _NWB], bf16, tag="wtb")
        wt3 = wpool.tile([128, _NW32], f32, tag="wt3")

        def w8s(name, rows=128):
            lo, hi = _W8[name]
            return wt8[0:rows, lo:hi]

        def wbs(name, rows=128):
            lo, hi = _WBF[name]
            return wtb[0:rows, lo:hi]

        def w3s(name, rows=128):
            lo, hi = _W32[name]
            return wt3[0:rows, lo:hi]

        sb = ctx.enter_context(tc.tile_pool(name="sb", bufs=2))
        ldp = ctx.enter_context(tc.tile_pool(name="ld", bufs=2))
        elp = ctx.enter_context(tc.tile_pool(name="el", bufs=NGRP))
        eyp = ctx.enter_context(tc.tile_pool(name="ey", bufs=NGRP))
        dvp = ctx.enter_context(tc.tile_pool(name="dv", bufs=NGRP))
        accp = ctx.enter_context(tc.tile_pool(name="acc", bufs=1))
        m4p = ctx.enter_context(tc.tile_pool(name="m4p", bufs=NGRP))
        outp = ctx.enter_context(tc.tile_pool(name="outs", bufs=3))
        bsp = ctx.enter_context(tc.tile_pool(name="bsp", bufs=NGRP))
        el_tiles, ey_tiles, m4_tiles = [], [], []
        accs = accp.tile([GRP, NGRP], f32)

        # ---------------- main loop: node + col(A) per group ----------
        with tc.tile_pool(name="h1pp", bufs=1, space="PSUM") as h1pp, \
             tc.tile_pool(name="h2lp", bufs=1, space="PSUM") as h2lp, \
             tc.tile_pool(name="h01pp", bufs=2, space="PSUM") as h01pp, \
             tc.tile_pool(name="h2cpp", bufs=1, space="PSUM") as h2cpp, \
             tc.tile_pool(name="ypp", bufs=1, space="PSUM") as ypp, \
             tc.tile_pool(name="l4pp", bufs=1, space="PSUM") as l4pp, \
             tc.tile_pool(name="d4pp", bufs=1, space="PSUM") as d4pp:
            mped = []
            for c in range(NGRP):
                w = GRP * CH
                off = GRP * CH * c
                xg2 = ldp.tile([128, w], f8, tag="xg2", name="xg2")
                nc.sync.dma_start(out=xg2[:], in_=xt_p[:, off:off + w])
                c0 = ldp.tile([128, w], f8, tag="c0", name="c0")
                nc.sync.dma_start(out=c0[:], in_=ct_p[0:128, off:off + w])
                if c == 0:
                    nc.sync.dma_start(out=wt3[:], in_=w3_p[:])
                    nc.sync.dma_start(out=wtb[:], in_=wb_p[:])
                    # pre-load the ACT spline table set (relu/exp/ln all in
                    # natural_log_exp_and_others) with no DMA dependency
                    warm = sb.tile([1, 2], f32, tag="warm")
                    nc.vector.memset(warm[:, 0:1], 0.0)
                    nc.scalar.activation(warm[:, 1:2], warm[:, 0:1], AF.Exp)
                c1 = ldp.tile([128, w], f8, tag="c1", name="c1")
                nc.sync.dma_start(out=c1[:], in_=ct_p[128:256, off:off + w])
                c2t = ldp.tile([128, w // 2], f8, tag="c2t", name="c2t")
                nc.sync.dma_start(out=c2t[:],
                                  in_=c2_p[:, off // 2:(off + w) // 2])
                xo = 0

                # --- node path ---
                h1p = h1pp.tile([128, 512], f32, tag="h1p", name="h1p")[:, 0:CH]
                for j in range(GRP):
                    nc.tensor.matmul(h1p[32 * j:32 * j + 32, :],
                                     lhsT=w8s("w1p"),
                                     rhs=xg2[:, xo + CH * j:xo + CH * (j + 1)],
                                     start=True, stop=True,
                                     tile_position=(0, 32 * j))
                h1s = sb.tile([128, CH], bf16, tag="h1s")
                nc.scalar.activation(h1s[:], h1p[:], AF.Relu, bias=w3s("b1r"))
                h2l = h2lp.tile([64, 512], f32, tag="h2l", name="h2l")[:, 0:CH]
                nc.tensor.matmul(h2l[:], lhsT=wbs("w2b"), rhs=h1s[:],
                                 start=True, stop=True)
                h2s = sb.tile([64, CH], bf16, tag="h2s")
                nc.scalar.activation(h2s[:], h2l[:], AF.Relu,
                                     bias=w3s("b2r", 64))
                l4p = l4pp.tile([GRP, 512], f32, tag="l4p",
                                name="l4p")[:, 0:CH]
                nc.tensor.matmul(l4p[:], lhsT=wbs("wfcb", 64),
                                 rhs=h2s[:], start=True, stop=True)
                el = elp.tile([GRP, CH], bf16, tag="el")
                nc.scalar.activation(el[:], l4p[:], AF.Exp,
                                     accum_out=accs[:, c:c + 1])

                # --- col path (softmax-independent part) ---
                h2cp = h2cpp.tile([128, 512], f32, tag="h2cp",
                                  name="h2cp")[:, 0:CH]
                for p in range(2):
                    nc.tensor.matmul(h2cp[64 * p:64 * p + 64, :],
                                     lhsT=w8s("wc1b2p"),
                                     rhs=c2t[:, xo // 2 + CH * p:
                                             xo // 2 + CH * (p + 1)],
                                     start=True, stop=True,
                                     tile_position=(0, 64 * p))
                h01s_tiles = []
                for j in range(GRP):
                    cs = slice(xo + CH * j, xo + CH * (j + 1))
                    h01p = h01pp.tile([128, 512], f32, tag="h01p",
                                      name="h01p")[:, 0:CH]
                    nc.tensor.matmul(h01p[0:64, :], lhsT=w8s("wc1b"),
                                     rhs=c0[:, cs], start=True, stop=True)
                    nc.tensor.matmul(h01p[64:128, :], lhsT=w8s("wc1b"),
                                     rhs=c1[:, cs], start=True, stop=True,
                                     tile_position=(0, 64))
                    h01s = sb.tile([128, CH], bf16, tag=f"h01s{j % 2}")
                    nc.vector.tensor_scalar(h01s[:], h01p[:], w3s("bc1r"),
                                            0.0, ALU.add, ALU.max)
                    h01s_tiles.append(h01s)
                h2cs = sb.tile([128, CH], bf16, tag="h2cs")
                nc.scalar.activation(h2cs[:], h2cp[:], AF.Relu,
                                     bias=w3s("bc1r"))
                yp = ypp.tile([128, 512], f32, tag="yp", name="yp")[:, 0:CH]
                for j in range(GRP):
                    nc.tensor.matmul(yp[32 * j:32 * j + 32, :],
                                     lhsT=wbs("wc2a"), rhs=h01s_tiles[j][:],
                                     start=True, stop=False,
                                     skip_group_check=True,
                                     tile_position=(0, 32 * j))
                for j in range(GRP):
                    nc.tensor.matmul(yp[32 * j:32 * j + 32, :],
                                     lhsT=wtb[32 * j:32 * j + 32,
                                              slice(*_WBF["wc2b"])],
                                     rhs=h2cs[32 * j:32 * j + 32, :],
                                     start=False, stop=True,
                                     skip_group_check=True,
                                     tile_position=(32 * j, 32 * j))
                ey = eyp.tile([128, CH], bf16, tag="ey")
                nc.scalar.activation(ey[:], yp[:], AF.Exp)
                ey_tiles.append(ey)
                d4p = d4pp.tile([GRP, 512], f32, tag="ps", name="d4p")[:, 0:CH]
                nc.tensor.matmul(d4p[:], lhsT=wbs("kmask"), rhs=ey[:],
                                 start=True, stop=True)
                lnD = sb.tile([GRP, CH], f32, tag="lnD")
                nc.scalar.activation(lnD[:], d4p[:], AF.Ln)
                dinv = dvp.tile([GRP, CH], bf16, tag="dinv")
                nc.scalar.activation(dinv[:], lnD[:], AF.Exp, scale=-1.0)
                m4 = m4p.tile([GRP, CH], bf16, tag="m4")
                nc.vector.tensor_mul(m4[:], el[:], dinv[:])
                m4_tiles.append(m4)

            # ---------- finalize: softmax normalization + output ------
            sp = d4pp.tile([2, 512], f32, tag="ps", name="sp")[:, 0:1]
            lo = _W32["smsk"][0]
            for c in range(NGRP):
                nc.tensor.matmul(sp[:],
                                 lhsT=wt3[0:GRP, lo + 2 * c:lo + 2 * c + 2],
                                 rhs=accs[:, c:c + 1],
                                 start=(c == 0), stop=(c == NGRP - 1),
                                 skip_group_check=True)
            sinv = sb.tile([2, 1], f32, tag="sinv")
            nc.vector.reciprocal(sinv[:], sp[:])
            s4p = d4pp.tile([GRP, 512], f32, tag="ps", name="s4p")[:, 0:NGRP]
            lo = _W32["smap"][0]
            for c in range(NGRP):
                nc.tensor.matmul(s4p[:, c:c + 1],
                                 lhsT=wt3[0:2, lo + 4 * c:lo + 4 * c + 4],
                                 rhs=sinv[:], start=True, stop=True,
                                 skip_group_check=True)
            s4s = sb.tile([GRP, NGRP], f32, tag="s4s")
            nc.vector.tensor_copy(s4s[:], s4p[:])
            for c in range(NGRP):
                bst = bsp.tile([GRP, 128], bf16, tag="bs")
                nc.vector.tensor_scalar_mul(bst[:], w3s("bmask", GRP),
                                            s4s[:, c:c + 1])
                mbp = d4pp.tile([128, 512], f32, tag="ps",
                                name="mbp")[:, 0:CH]
                nc.tensor.matmul(mbp[:], lhsT=bst[:], rhs=m4_tiles[c][:],
                                 start=True, stop=True)
                ot = outp.tile([128, CH], f32, tag="ot")
                nc.vector.tensor_mul(ot[:], ey_tiles[c][:], mbp[:])
                nc.gpsimd.dma_start(out=out_p[:, CH * c:CH * (c + 1)],
                                    in_=ot[:])

    nc.compile()
    bacc_mod.get_activation_tables = _orig_gat
    return nc


def _get_compiled():
    global _CACHED
    if _CACHED is None:
        _CACHED = _build()
    return _CACHED


def _prep_inputs(node_features, col_features, W1, b1, W2, b2, Wfc,
                 Wc1, bc1, Wc2):
    import ml_dtypes
    f8 = ml_dtypes.float8_e4m3
    nf = np.asarray(node_features, np.float32)
    cf = np.asarray(col_features, np.float32)
    xt = np.ascontiguousarray(
        nf.reshape(NCORES, GPC, N, F).transpose(0, 3, 1, 2)
        .reshape(NCORES, F, M)).astype(f8)
    ctf = np.ascontiguousarray(
        cf.reshape(NCORES, GPC, N, K, FC).transpose(0, 3, 4, 1, 2)
        .reshape(NCORES, K * FC, M)).astype(f8)
    ct = np.ascontiguousarray(ctf[:, 0:256])
    # k = 8,9 rows, chunk-paired: [parity, 64 rows, pairs, 500] -> [128, M/2]
    c2 = np.ascontiguousarray(
        ctf[:, 256:320].reshape(NCORES, 64, M // (2 * CH), 2, CH)
        .transpose(0, 3, 1, 2, 4).reshape(NCORES, 128, M // 2))
    w8, wb, w3 = _pack_consts(
        np.asarray(W1, np.float32), np.asarray(b1, np.float32),
        np.asarray(W2, np.float32), np.asarray(b2, np.float32),
        np.asarray(Wfc, np.float32), np.asarray(Wc1, np.float32),
        np.asarray(bc1, np.float32), np.asarray(Wc2, np.float32))
    return xt, ct, c2, w8, wb, w3


def kernel(node_features, col_features, edge_index=None,
           W1=None, b1=None, W2=None, b2=None, Wfc=None, bfc=None,
           Wc1=None, bc1=None, Wc2=None, bc2=None, **_unused):
    from concourse.bass_utils import run_bass_kernel_spmd

    xt, ct, c2, w8, wb, w3 = _prep_inputs(node_features, col_features,
                                          W1, b1, W2, b2, Wfc, Wc1, bc1, Wc2)
    nc = _get_compiled()
    in_maps = [{"xt": xt[i], "ct": ct[i], "c2": c2[i],
                "w8": w8, "wb": wb, "w3": w3} for i in range(NCORES)]
    res = run_bass_kernel_spmd(nc, in_maps, core_ids=list(range(NCORES)))
    outs = np.stack([res.results[i]["out"] for i in range(NCORES)])
    # outs[i][32j+k, 500c+nn] = value for node 2000c+500j+nn, class k
    o = outs.reshape(NCORES, 4, 32, NGRP, CH)[:, :, 0:K]   # [i, j, k, c, nn]
    o = o.transpose(0, 3, 1, 4, 2)                         # [i, c, j, nn, k]
    out = o.reshape(NCORES, GPC, N, K).reshape(B, N * K)
    return np.ascontiguousarray(out)
